# revision 4
# baseline (speedup 1.0000x reference)
"""Trainium2 Bass kernel for nn_CrossAttention (B=2, S=2048, E=1024, H=16, ctx=768).

Sharding: 4-way tensor-parallel over heads x 2-way data-parallel over batch.
Core c handles batch c//4 and heads 4*(c%4) .. 4*(c%4)+3.

Per-core dataflow (all matmuls fp16 operands, fp32 PSUM accumulate):
  qT/kT = W.T-stationary projections producing [dh, S] layouts directly
  v     = ctxT-tile-stationary projection producing natural [S, dh]
  scT   = kT-tile x qT (K=64, two heads row-packed in the PE array)
  exp   = ScalarE, fused 1/sqrt(dh) scale, PSUM -> SBUF fp16
  av/Z  = [v_h | ones] col-packed: PSUM rows 0:64 = unnormalized out.T,
          rows 64:128 = softmax denominator replicated; normalize on DVE
  out   = avT-stationary x Wo, partial [S, E] per core

Host side: pre-transpose x/context, slice weights per head group, fp16 cast;
sum the 4 per-batch partials + bo on host.
"""
import numpy as np

import concourse.bass as bass
import concourse.mybir as mybir
import concourse.tile as tile
from concourse import bacc, bass_utils

F16 = mybir.dt.float16
F32 = mybir.dt.float32
AF = mybir.ActivationFunctionType
OP = mybir.AluOpType

B, S, E, C, H, DH = 2, 2048, 1024, 768, 16, 64
N_CORES = 8
GROUPS = 4            # head groups (tensor parallel)
HPG = H // GROUPS     # heads per group = 4
DSL = HPG * DH        # feature slice per core = 256
KT_E = E // 128       # 8 k-tiles for x projections
KT_C = C // 128       # 6 k-tiles for context projections
SCK = S // 512        # 4 s-chunks
TT = S // 128         # 16 t-tiles

_NC_CACHE = {}


def _build_nc():
    nc = bacc.Bacc("TRN2", target_bir_lowering=False, debug=False,
                   num_devices=N_CORES)

    xT = nc.dram_tensor("xT", [E, S], F16, kind="ExternalInput").ap()
    ctxT = nc.dram_tensor("ctxT", [C, S], F16, kind="ExternalInput").ap()
    wq = nc.dram_tensor("wq", [E, DSL], F16, kind="ExternalInput").ap()
    wk = nc.dram_tensor("wk", [C, DSL], F16, kind="ExternalInput").ap()
    wv = nc.dram_tensor("wv", [C, DSL], F16, kind="ExternalInput").ap()
    wo = nc.dram_tensor("wo", [DSL, E], F16, kind="ExternalInput").ap()
    bq = nc.dram_tensor("bq", [128, 2], F32, kind="ExternalInput").ap()
    bk = nc.dram_tensor("bk", [128, 2], F32, kind="ExternalInput").ap()
    bv = nc.dram_tensor("bv", [1, DSL], F16, kind="ExternalInput").ap()
    out = nc.dram_tensor("out", [S, E], F32, kind="ExternalOutput").ap()

    with tile.TileContext(nc) as tc:
        with (
            tc.tile_pool(name="const", bufs=1) as cpool,
            tc.tile_pool(name="qkv", bufs=1) as qpool,
            tc.tile_pool(name="ex", bufs=3) as expool,
            tc.tile_pool(name="rz", bufs=2) as rzpool,
            tc.tile_pool(name="os", bufs=3) as ospool,
        ):
            xT_sb = cpool.tile([128, KT_E, S], F16)
            ctxT_sb = cpool.tile([128, KT_C, S], F16)
            wq_sb = cpool.tile([128, KT_E, DSL], F16)
            wk_sb = cpool.tile([128, KT_C, DSL], F16)
            wv_sb = cpool.tile([128, KT_C, DSL], F16)
            wo_sb = cpool.tile([128, 2, E], F16)
            bq_sb = cpool.tile([128, 2], F32)
            bk_sb = cpool.tile([128, 2], F32)
            bv_sb = cpool.tile([1, DSL], F16)
            ones_sb = cpool.tile([128, DH], F16)
            onesr_sb = cpool.tile([1, 128], F16)

            nc.sync.dma_start(xT_sb[:], xT.rearrange("(o p) s -> p o s", p=128))
            nc.sync.dma_start(ctxT_sb[:], ctxT.rearrange("(o p) s -> p o s", p=128))
            nc.sync.dma_start(wq_sb[:], wq.rearrange("(o p) m -> p o m", p=128))
            nc.sync.dma_start(wk_sb[:], wk.rearrange("(o p) m -> p o m", p=128))
            nc.sync.dma_start(wv_sb[:], wv.rearrange("(o p) m -> p o m", p=128))
            nc.sync.dma_start(wo_sb[:], wo.rearrange("(l p) n -> p l n", p=128))
            nc.sync.dma_start(bq_sb[:], bq[:])
            nc.sync.dma_start(bk_sb[:], bk[:])
            nc.sync.dma_start(bv_sb[:], bv[:])
            nc.vector.memset(ones_sb[:], 1.0)
            nc.vector.memset(onesr_sb[:], 1.0)

            qT_sb = qpool.tile([128, 2, S], F16)
            kT_sb = qpool.tile([128, 2, S], F16)
            v_sb = qpool.tile([128, TT, DSL], F16)
            avT_sb = qpool.tile([128, 2, S], F16)

            # ---- projections ----
            with (
                tc.tile_pool(name="pqk", bufs=2, space="PSUM") as pqk,
                tc.tile_pool(name="pv", bufs=2, space="PSUM") as pv,
            ):
                for dst, w_sb, b_sb, src, nk in (
                    (qT_sb, wq_sb, bq_sb, xT_sb, KT_E),
                    (kT_sb, wk_sb, bk_sb, ctxT_sb, KT_C),
                ):
                    for l in range(2):
                        for sc in range(SCK):
                            ps = pqk.tile([128, 512], F32, tag="qk")
                            for k in range(nk):
                                nc.tensor.matmul(
                                    ps[:],
                                    w_sb[:, k, l * 128:(l + 1) * 128],
                                    src[:, k, sc * 512:(sc + 1) * 512],
                                    start=(k == 0), stop=(k == nk - 1),
                                )
                            nc.vector.tensor_tensor(
                                dst[:, l, sc * 512:(sc + 1) * 512],
                                ps[:],
                                b_sb[:, l:l + 1].to_broadcast([128, 512]),
                                OP.add,
                            )

                for t in range(TT):
                    ps = pv.tile([128, DSL], F32, tag="v")
                    for k in range(KT_C):
                        nc.tensor.matmul(
                            ps[:],
                            ctxT_sb[:, k, t * 128:(t + 1) * 128],
                            wv_sb[:, k, :],
                            start=(k == 0), stop=False,
                        )
                    nc.tensor.matmul(
                        ps[:], onesr_sb[:, :], bv_sb[:, :],
                        start=False, stop=True,
                    )
                    nc.vector.tensor_copy(v_sb[:, t, :], ps[:])

            # ---- attention + output projection ----
            with (
                tc.tile_pool(name="psc", bufs=2, space="PSUM") as psc,
                tc.tile_pool(name="pavz", bufs=2, space="PSUM") as pavz,
                tc.tile_pool(name="po", bufs=2, space="PSUM") as po,
            ):
                for sc in range(SCK):
                    for p in range(2):
                        avz = [pavz.tile([128, 512], F32, tag="avz",
                                         name=f"avz{sc}_{p}_{i}")
                               for i in range(2)]
                        for tg in range(TT // 2):
                            for h in range(2):
                                hb = h * DH
                                scps = psc.tile([128, 1024], F32, tag="sc")
                                for j in range(2):
                                    t = tg * 2 + j
                                    nc.tensor.matmul(
                                        scps[:, j * 512:(j + 1) * 512],
                                        kT_sb[hb:hb + DH, p, t * 128:(t + 1) * 128],
                                        qT_sb[hb:hb + DH, p, sc * 512:(sc + 1) * 512],
                                        start=True, stop=True,
                                    )
                                ex = expool.tile([128, 1024], F16, tag="ex")
                                nc.scalar.activation(ex[:], scps[:], AF.Exp,
                                                     scale=0.125)
                                for j in range(2):
                                    t = tg * 2 + j
                                    nc.tensor.matmul(
                                        avz[h][0:DH, :],
                                        v_sb[:, t, p * 128 + hb:p * 128 + hb + DH],
                                        ex[:, j * 512:(j + 1) * 512],
                                        start=(t == 0), stop=(t == TT - 1),
                                        skip_group_check=True,
                                    )
                                    nc.tensor.matmul(
                                        avz[h][DH:128, :],
                                        ones_sb[:, :],
                                        ex[:, j * 512:(j + 1) * 512],
                                        start=(t == 0), stop=(t == TT - 1),
                                        skip_group_check=True,
                                    )
                        for h in range(2):
                            hb = h * DH
                            rz = rzpool.tile([128, 512], F32, tag="rz")
                            nc.vector.reciprocal(rz[0:DH, :], avz[h][DH:128, :])
                            nc.vector.tensor_tensor(
                                avT_sb[hb:hb + DH, p, sc * 512:(sc + 1) * 512],
                                avz[h][0:DH, :],
                                rz[0:DH, :],
                                OP.mult,
                            )
                    # output projection for this s-chunk
                    for st in range(4):
                        row = (sc * 4 + st) * 128
                        for n in range(2):
                            pso = po.tile([128, 512], F32, tag="o")
                            for l in range(2):
                                nc.tensor.matmul(
                                    pso[:],
                                    avT_sb[:, l, row:row + 128],
                                    wo_sb[:, l, n * 512:(n + 1) * 512],
                                    start=(l == 0), stop=(l == 1),
                                )
                            os_sb = ospool.tile([128, 512], F32, tag="os")
                            nc.vector.tensor_copy(os_sb[:], pso[:])
                            nc.sync.dma_start(
                                out[row:row + 128, n * 512:(n + 1) * 512],
                                os_sb[:],
                            )

    nc.compile()
    return nc


def get_nc():
    if "nc" not in _NC_CACHE:
        _NC_CACHE["nc"] = _build_nc()
    return _NC_CACHE["nc"]


def make_in_maps(x, context, Wq, bq, Wk, bk, Wv, bv, Wo, bo):
    x = np.asarray(x, dtype=np.float32)
    context = np.asarray(context, dtype=np.float32)
    Wq = np.asarray(Wq, dtype=np.float32)
    Wk = np.asarray(Wk, dtype=np.float32)
    Wv = np.asarray(Wv, dtype=np.float32)
    Wo = np.asarray(Wo, dtype=np.float32)
    bq = np.asarray(bq, dtype=np.float32)
    bk = np.asarray(bk, dtype=np.float32)
    bv = np.asarray(bv, dtype=np.float32)

    xT = [np.ascontiguousarray(x[b].T).astype(np.float16) for b in range(B)]
    ctxT = [np.ascontiguousarray(context[b].T).astype(np.float16)
            for b in range(B)]
    in_maps = []
    for c in range(N_CORES):
        b, g = c // GROUPS, c % GROUPS
        sl = slice(g * DSL, (g + 1) * DSL)
        in_maps.append({
            "xT": xT[b],
            "ctxT": ctxT[b],
            "wq": Wq[:, sl].astype(np.float16),
            "wk": Wk[:, sl].astype(np.float16),
            "wv": Wv[:, sl].astype(np.float16),
            "wo": Wo[sl, :].astype(np.float16),
            "bq": np.ascontiguousarray(bq[sl].reshape(2, 128).T),
            "bk": np.ascontiguousarray(bk[sl].reshape(2, 128).T),
            "bv": bv[sl].reshape(1, DSL).astype(np.float16),
        })
    return in_maps


def run_sharded(inputs, trace=False):
    nc = get_nc()
    in_maps = make_in_maps(**inputs)
    res = bass_utils.run_bass_kernel_spmd(
        nc, in_maps, core_ids=list(range(N_CORES)), trace=trace,
    )
    bo = np.asarray(inputs["bo"], dtype=np.float32)
    full = np.empty((B, S, E), dtype=np.float32)
    for b in range(B):
        acc = res.results[b * GROUPS]["out"].astype(np.float32)
        for g in range(1, GROUPS):
            acc = acc + res.results[b * GROUPS + g]["out"]
        full[b] = acc + bo[None, :]
    return full, res.exec_time_ns


def kernel(**inputs) -> np.ndarray:
    return run_sharded(inputs)[0]


# revision 12
# speedup vs baseline: 1.2685x; 1.2685x over previous
"""Trainium2 Bass kernel for nn_CrossAttention (B=2, S=2048, E=1024, H=16, ctx=768).

Sharding: 4-way tensor-parallel over heads x 2-way data-parallel over batch.
Core c handles batch c//4 and heads 4*(c%4) .. 4*(c%4)+3.

Per-core dataflow (all matmuls fp16 operands, fp32 PSUM accumulate):
  qT/kT = W-stationary projections producing [dh, S] layouts directly
  v     = ctxT-tile-stationary projection producing natural [S, dh]
  scT   = kT-tile x qT (K=64); the two heads of a pair are emitted
          back-to-back on PE row groups 0/64 so they run concurrently
  exp   = ScalarE, fused 1/sqrt(dh) scale, PSUM -> SBUF fp16
  av/Z  = v_h (cols 0:64) and ones (cols 64:128) col-packed into one
          PSUM bank: rows 0:64 = unnormalized out.T, rows 64:128 =
          softmax denominator replicated; normalized via DVE divide
  out   = avT-stationary x Wo, partial [S, E] per core

Host side: pre-transpose x/context, slice weights per head group, fp16 cast;
sum the 4 per-batch partials + bo on host.
"""
import os
import numpy as np

import concourse.bass as bass
import concourse.mybir as mybir
import concourse.tile as tile
from concourse import bacc, bass_utils

F16 = mybir.dt.float16
F32 = mybir.dt.float32
AF = mybir.ActivationFunctionType
OP = mybir.AluOpType

B, S, E, C, H, DH = 2, 2048, 1024, 768, 16, 64
N_CORES = 8
GROUPS = 4            # head groups (tensor parallel)
HPG = H // GROUPS     # heads per group = 4
DSL = HPG * DH        # feature slice per core = 256
KT_E = E // 128       # 8 k-tiles for x projections
KT_C = C // 128       # 6 k-tiles for context projections
SCK = S // 512        # 4 s-chunks
TT = S // 128         # 16 t-tiles

NORM_MODE = os.environ.get("NORM_MODE", "recipfast")
SC_PAIR = os.environ.get("SC_PAIR", "1") == "1"
PO_SHARE = os.environ.get("PO_SHARE", "1") == "1"

_NC_CACHE = {}


def _build_nc():
    nc = bacc.Bacc("TRN2", target_bir_lowering=False, debug=False,
                   num_devices=N_CORES)

    xT = nc.dram_tensor("xT", [E, S], F16, kind="ExternalInput").ap()
    ctxT = nc.dram_tensor("ctxT", [C, S], F16, kind="ExternalInput").ap()
    wq = nc.dram_tensor("wq", [E, DSL], F16, kind="ExternalInput").ap()
    wk = nc.dram_tensor("wk", [C, DSL], F16, kind="ExternalInput").ap()
    wv = nc.dram_tensor("wv", [C, DSL], F16, kind="ExternalInput").ap()
    wo = nc.dram_tensor("wo", [DSL, E], F16, kind="ExternalInput").ap()
    bq = nc.dram_tensor("bq", [128, 2], F32, kind="ExternalInput").ap()
    bk = nc.dram_tensor("bk", [128, 2], F32, kind="ExternalInput").ap()
    bv = nc.dram_tensor("bv", [1, DSL], F16, kind="ExternalInput").ap()
    out = nc.dram_tensor("out", [S, E], F32, kind="ExternalOutput").ap()

    xT_r = xT.rearrange("(o p) s -> p o s", p=128)
    ctxT_r = ctxT.rearrange("(o p) s -> p o s", p=128)

    with tile.TileContext(nc) as tc:
        with (
            tc.tile_pool(name="const", bufs=1) as cpool,
            tc.tile_pool(name="qkv", bufs=1) as qpool,
            tc.tile_pool(name="ex", bufs=4) as expool,
            tc.tile_pool(name="os", bufs=3) as ospool,
        ):
            wq_sb = cpool.tile([128, KT_E, DSL], F16)
            wk_sb = cpool.tile([128, KT_C, DSL], F16)
            wv_sb = cpool.tile([128, KT_C, DSL], F16)
            wo_sb = cpool.tile([128, 2, E], F16)
            bq_sb = cpool.tile([128, 2], F32)
            bk_sb = cpool.tile([128, 2], F32)
            bv_sb = cpool.tile([1, DSL], F16)
            ones_sb = cpool.tile([128, DH], F16)
            onesr_sb = cpool.tile([1, 128], F16)
            warm_sb = cpool.tile([1, 8], F32)
            ctxT_sb = cpool.tile([128, KT_C, S], F16)
            xT_sb = cpool.tile([128, KT_E, S], F16)

            nc.sync.dma_start(wk_sb[:], wk.rearrange("(o p) m -> p o m", p=128))
            nc.sync.dma_start(wv_sb[:], wv.rearrange("(o p) m -> p o m", p=128))
            nc.sync.dma_start(wq_sb[:], wq.rearrange("(o p) m -> p o m", p=128))
            nc.sync.dma_start(wo_sb[:], wo.rearrange("(l p) n -> p l n", p=128))
            nc.sync.dma_start(bq_sb[:], bq[:])
            nc.sync.dma_start(bk_sb[:], bk[:])
            nc.sync.dma_start(bv_sb[:], bv[:])
            nc.vector.memset(ones_sb[:], 1.0)
            nc.vector.memset(onesr_sb[:], 1.0)
            nc.vector.memset(warm_sb[:], 0.0)
            # pull the exp table load off the critical path
            nc.scalar.activation(warm_sb[:], warm_sb[:], AF.Exp)
            for k in range(KT_C):
                nc.sync.dma_start(ctxT_sb[:, k, :], ctxT_r[:, k, :])
            for k in range(KT_E):
                nc.sync.dma_start(xT_sb[:, k, :], xT_r[:, k, :])

            qT_sb = qpool.tile([128, 2, S], F16)
            kT_sb = qpool.tile([128, 2, S], F16)
            v_sb = qpool.tile([128, TT, DSL], F16)
            avT_sb = qpool.tile([128, 2, S], F16)

            # ---- projections (k, v first: attention needs them complete) ----
            with (
                tc.tile_pool(name="pqk", bufs=4, space="PSUM") as pqk,
                tc.tile_pool(name="pv", bufs=2, space="PSUM") as pv,
            ):
                def proj_qk(dst, w_sb, b_sb, src, nk):
                    for l in range(2):
                        for sc in range(SCK):
                            ps = pqk.tile([128, 512], F32, tag="qk")
                            for k in range(nk):
                                nc.tensor.matmul(
                                    ps[:],
                                    w_sb[:, k, l * 128:(l + 1) * 128],
                                    src[:, k, sc * 512:(sc + 1) * 512],
                                    start=(k == 0), stop=(k == nk - 1),
                                )
                            nc.vector.tensor_tensor(
                                dst[:, l, sc * 512:(sc + 1) * 512],
                                ps[:],
                                b_sb[:, l:l + 1].to_broadcast([128, 512]),
                                OP.add,
                            )

                proj_qk(kT_sb, wk_sb, bk_sb, ctxT_sb, KT_C)

                for t in range(TT):
                    ps = pv.tile([128, DSL], F32, tag="v")
                    for k in range(KT_C):
                        nc.tensor.matmul(
                            ps[:],
                            ctxT_sb[:, k, t * 128:(t + 1) * 128],
                            wv_sb[:, k, :],
                            start=(k == 0), stop=False,
                        )
                    nc.tensor.matmul(
                        ps[:], onesr_sb[:, :], bv_sb[:, :],
                        start=False, stop=True,
                    )
                    nc.vector.tensor_copy(v_sb[:, t, :], ps[:])

                proj_qk(qT_sb, wq_sb, bq_sb, xT_sb, KT_E)

            # ---- attention + output projection ----
            import contextlib
            with (
                tc.tile_pool(name="psc", bufs=(3 if PO_SHARE else 2),
                             space="PSUM") as psc,
                tc.tile_pool(name="pavz", bufs=2, space="PSUM") as pavz,
                (contextlib.nullcontext(psc) if PO_SHARE else
                 tc.tile_pool(name="po", bufs=2, space="PSUM")) as popool,
            ):
                for sc in range(SCK):
                    ssl = slice(sc * 512, (sc + 1) * 512)
                    for p in range(2):
                        avz = [pavz.tile([128, 512], F32, tag="avz",
                                         name=f"avz{sc}_{p}_{i}")
                               for i in range(2)]
                        for tg in range(TT // 2):
                            if SC_PAIR:
                                scp = [psc.tile([128, 1024], F32, tag="sc",
                                                name=f"sc{sc}_{p}_{tg}_{i}")
                                       for i in range(2)]
                                # paired heads on PE row groups 0 / 64
                                for j in range(2):
                                    t = tg * 2 + j
                                    for h in range(2):
                                        hb = h * DH
                                        nc.tensor.matmul(
                                            scp[h][:, j * 512:(j + 1) * 512],
                                            kT_sb[hb:hb + DH, p, t * 128:(t + 1) * 128],
                                            qT_sb[hb:hb + DH, p, ssl],
                                            start=True, stop=True,
                                        )
                                exs = []
                                for h in range(2):
                                    ex = expool.tile([128, 1024], F16, tag="ex",
                                                     name=f"ex{sc}_{p}_{tg}_{h}")
                                    nc.scalar.activation(ex[:], scp[h][:], AF.Exp,
                                                         scale=0.125)
                                    exs.append(ex)
                                for j in range(2):
                                    t = tg * 2 + j
                                    jsl = slice(j * 512, (j + 1) * 512)
                                    for h in range(2):
                                        hb = h * DH
                                        nc.tensor.matmul(
                                            avz[h][0:DH, :],
                                            v_sb[:, t, p * 128 + hb:p * 128 + hb + DH],
                                            exs[h][:, jsl],
                                            start=(t == 0), stop=(t == TT - 1),
                                            skip_group_check=True,
                                        )
                                        nc.tensor.matmul(
                                            avz[h][DH:128, :],
                                            ones_sb[:, :],
                                            exs[h][:, jsl],
                                            start=(t == 0), stop=(t == TT - 1),
                                            skip_group_check=True,
                                        )
                            else:
                                for h in range(2):
                                    hb = h * DH
                                    scps = psc.tile([128, 1024], F32, tag="sc",
                                                    name=f"sc{sc}_{p}_{tg}_{h}")
                                    for j in range(2):
                                        t = tg * 2 + j
                                        nc.tensor.matmul(
                                            scps[:, j * 512:(j + 1) * 512],
                                            kT_sb[hb:hb + DH, p, t * 128:(t + 1) * 128],
                                            qT_sb[hb:hb + DH, p, ssl],
                                            start=True, stop=True,
                                        )
                                    ex = expool.tile([128, 1024], F16, tag="ex",
                                                     name=f"ex{sc}_{p}_{tg}_{h}")
                                    nc.scalar.activation(ex[:], scps[:], AF.Exp,
                                                         scale=0.125)
                                    for j in range(2):
                                        t = tg * 2 + j
                                        jsl = slice(j * 512, (j + 1) * 512)
                                        nc.tensor.matmul(
                                            avz[h][0:DH, :],
                                            v_sb[:, t, p * 128 + hb:p * 128 + hb + DH],
                                            ex[:, jsl],
                                            start=(t == 0), stop=(t == TT - 1),
                                            skip_group_check=True,
                                        )
                                        nc.tensor.matmul(
                                            avz[h][DH:128, :],
                                            ones_sb[:, :],
                                            ex[:, jsl],
                                            start=(t == 0), stop=(t == TT - 1),
                                            skip_group_check=True,
                                        )
                        for h in range(2):
                            hb = h * DH
                            if NORM_MODE == "divide":
                                nc.vector.tensor_tensor(
                                    avT_sb[hb:hb + DH, p, ssl],
                                    avz[h][0:DH, :],
                                    avz[h][DH:128, :],
                                    OP.divide,
                                )
                            else:
                                # recipfast is a custom DVE op: SBUF-only and
                                # partition-base-shift intolerant. Stage Z to
                                # SBUF unshifted, recip unshifted, then a
                                # standard mixed-base multiply.
                                rz = ospool.tile([128, 1024], F32, tag="rz",
                                                 name=f"rz{sc}_{p}_{h}")
                                nc.vector.tensor_copy(
                                    rz[0:DH, 0:512], avz[h][DH:128, :])
                                nc.vector.reciprocal_approx_fast(
                                    rz[0:DH, 512:1024], rz[0:DH, 0:512])
                                nc.vector.tensor_tensor(
                                    avT_sb[hb:hb + DH, p, ssl],
                                    avz[h][0:DH, :],
                                    rz[0:DH, 512:1024],
                                    OP.mult,
                                )
                    # output projection for this s-chunk (psum shares sc pool)
                    for st in range(4):
                        row = (sc * 4 + st) * 128
                        for n in range(2):
                            pso = popool.tile([128, 512], F32,
                                              tag=("sc" if PO_SHARE else "o"),
                                              name=f"po{sc}_{st}_{n}")
                            for l in range(2):
                                nc.tensor.matmul(
                                    pso[:],
                                    avT_sb[:, l, row:row + 128],
                                    wo_sb[:, l, n * 512:(n + 1) * 512],
                                    start=(l == 0), stop=(l == 1),
                                )
                            os_sb = ospool.tile([128, 512], F32, tag="os")
                            nc.vector.tensor_copy(os_sb[:], pso[:])
                            nc.sync.dma_start(
                                out[row:row + 128, n * 512:(n + 1) * 512],
                                os_sb[:],
                            )

    nc.compile()
    return nc


def get_nc():
    if "nc" not in _NC_CACHE:
        _NC_CACHE["nc"] = _build_nc()
    return _NC_CACHE["nc"]


def make_in_maps(x, context, Wq, bq, Wk, bk, Wv, bv, Wo, bo):
    x = np.asarray(x, dtype=np.float32)
    context = np.asarray(context, dtype=np.float32)
    Wq = np.asarray(Wq, dtype=np.float32)
    Wk = np.asarray(Wk, dtype=np.float32)
    Wv = np.asarray(Wv, dtype=np.float32)
    Wo = np.asarray(Wo, dtype=np.float32)
    bq = np.asarray(bq, dtype=np.float32)
    bk = np.asarray(bk, dtype=np.float32)
    bv = np.asarray(bv, dtype=np.float32)

    xT = [np.ascontiguousarray(x[b].T).astype(np.float16) for b in range(B)]
    ctxT = [np.ascontiguousarray(context[b].T).astype(np.float16)
            for b in range(B)]
    in_maps = []
    for c in range(N_CORES):
        b, g = c // GROUPS, c % GROUPS
        sl = slice(g * DSL, (g + 1) * DSL)
        in_maps.append({
            "xT": xT[b],
            "ctxT": ctxT[b],
            "wq": Wq[:, sl].astype(np.float16),
            "wk": Wk[:, sl].astype(np.float16),
            "wv": Wv[:, sl].astype(np.float16),
            "wo": Wo[sl, :].astype(np.float16),
            "bq": np.ascontiguousarray(bq[sl].reshape(2, 128).T),
            "bk": np.ascontiguousarray(bk[sl].reshape(2, 128).T),
            "bv": bv[sl].reshape(1, DSL).astype(np.float16),
        })
    return in_maps


def run_sharded(inputs, trace=False):
    nc = get_nc()
    in_maps = make_in_maps(**inputs)
    res = bass_utils.run_bass_kernel_spmd(
        nc, in_maps, core_ids=list(range(N_CORES)), trace=trace,
    )
    bo = np.asarray(inputs["bo"], dtype=np.float32)
    full = np.empty((B, S, E), dtype=np.float32)
    for b in range(B):
        acc = res.results[b * GROUPS]["out"].astype(np.float32)
        for g in range(1, GROUPS):
            acc = acc + res.results[b * GROUPS + g]["out"]
        full[b] = acc + bo[None, :]
    return full, res.exec_time_ns


def kernel(**inputs) -> np.ndarray:
    return run_sharded(inputs)[0]
